# revision 6
# baseline (speedup 1.0000x reference)
# Distributed Bass kernel: causal multi-head attention block on 8 TRN2 NeuronCores.
#
# Problem (hardcoded): x [2, 4096, 768] f32, 12 heads x 64 dim, causal attention,
#   out = softmax(mask(q k^T / 8)) v  projected by Wo, all nn.Linear with bias.
#
# Sharding: core c -> batch b = c // 4, head-group hg = c % 4 (3 heads each).
#   Per core: QKV for its 3 heads over the full sequence (tensor parallel on
#   heads), flash-style causal attention, chunked AllGathers of preout^T
#   (bf16) within each 4-core batch group -- pipelined behind attention --
#   then an output projection sharded over dout (each core computes its own
#   192 output columns, written transposed [192, 4096] and flipped on host).
#
# Key layout/perf choices (v1):
#   - logits computed TRANSPOSED ([sj, si]) so exp() output a^T feeds the a@v
#     matmul with no transpose; v carries a leading ones column per head so
#     the same matmul accumulates the softmax denominator into po row 0.
#   - QK^T matmuls have contraction K=64 (head dim) -> only half the PE rows.
#     Fix: PE row-tiling. qT/kT live as [128, 2, S] tiles: partitions 0-63
#     hold heads {0, 2}, partitions 64-127 hold heads {1, 2-copy}. Each pair
#     of QK matmuls issues to tile_position (0,0) and (64,0) and runs
#     CONCURRENTLY on the two halves of the PE array (~2x QK throughput).
#     Heads 0/1 pair per-j; head 2 pairs j-parity (even j lo, odd j hi).
#   - v bias folded into v65 values (softmax weights sum to 1 post-normalize,
#     so (sum a (v+b))/denom == preout + b) -- no post-softmax bias add.
#   - x^T produced via PE transpose; the 6 per-128-row-block PSUM->SBUF
#     drains are batched into one DVE copy via a [128, 6, 128] PSUM tile.
#   - x f32->bf16 casts run on GpSimd (DVE is the phase-A bottleneck).
#   - last superchunk's AllGather is split per-head so the tail only waits
#     for head 2's small [64, 512] gather; output projection for that chunk
#     uses a head-permuted copy of Wo^T (wo7_bf).

import numpy as np

B = 2
S = 4096
D = 768
HD = 64
NH = 12
NCORES = 8
HL = 3            # heads per core
DL = HL * HD      # 192: local q/k/v dims per core
SUP = 512         # si superchunk
NSUP = S // SUP   # 8
NKC = S // 128    # 32 sj chunks
NDC = D // 128    # 6 contraction chunks
GROUPS = [[0, 1, 2, 3], [4, 5, 6, 7]]
CW = SUP          # AllGather chunk width

_CACHE = {}


def _build_nc():
    import concourse.mybir as mybir
    from concourse import bacc
    from concourse.tile import TileContext
    from concourse.masks import make_identity

    f32 = mybir.dt.float32
    bf16 = mybir.dt.bfloat16
    EXP = mybir.ActivationFunctionType.Exp

    nc = bacc.Bacc(num_devices=NCORES)

    x_p = nc.declare_dram_parameter("x", [S, D], f32, isOutput=False)
    wqk_p = nc.declare_dram_parameter("wqk", [2 * DL, D], f32, isOutput=False)
    bqk_p = nc.declare_dram_parameter("bqk", [2 * DL, 1], f32, isOutput=False)
    wv_p = nc.declare_dram_parameter("wv", [DL, D], f32, isOutput=False)
    bv_p = nc.declare_dram_parameter("bv", [DL, 1], f32, isOutput=False)
    wo_p = nc.declare_dram_parameter("wo", [DL, D], f32, isOutput=False)
    bo_p = nc.declare_dram_parameter("bo", [DL, 1], f32, isOutput=False)
    out_p = nc.declare_dram_parameter("out", [DL, S], f32, isOutput=True)

    cins = [nc.dram_tensor(f"cc_in{c}", [DL, CW], bf16) for c in range(NSUP - 1)]
    couts = [nc.dram_tensor(f"cc_out{c}", [D, CW], bf16) for c in range(NSUP - 1)]
    cins7 = [nc.dram_tensor(f"c7in{h}", [HD, CW], bf16) for h in range(HL)]
    couts7 = [nc.dram_tensor(f"c7out{h}", [4 * HD, CW], bf16) for h in range(HL)]

    with TileContext(nc) as tc:
        with (
            tc.tile_pool(name="const", bufs=1) as cpool,
            tc.tile_pool(name="wstage", bufs=2) as wstage,
            tc.tile_pool(name="xstage", bufs=3) as xstage,
            tc.tile_pool(name="at", bufs=3) as atpool,
            tc.tile_pool(name="ps", bufs=2) as pspool,
            tc.tile_pool(name="bc", bufs=2) as bcpool,
            tc.tile_pool(name="ot", bufs=2) as otpool,
            tc.tile_pool(name="mm", bufs=2, space="PSUM") as mmpsum,
            tc.tile_pool(name="lg", bufs=2, space="PSUM") as lgpsum,
            tc.tile_pool(name="po", bufs=2, space="PSUM") as popsum,
        ):
            # ---------------- constants / weights ----------------
            ident = cpool.tile([128, 128], bf16, name="ident")
            make_identity(nc, ident[:, :])

            # multiplicative causal masks for the 4 diagonal sj-chunk offsets:
            # masks[p, k, f] = 1.0 if (f - p - 128k) >= 0 else 0.0
            masks = cpool.tile([128, 4, SUP], bf16, name="masks")
            nc.gpsimd.memset(masks[:, :, :], 1.0)
            for k in range(4):
                nc.gpsimd.affine_select(
                    out=masks[:, k, :],
                    in_=masks[:, k, :],
                    compare_op=mybir.AluOpType.is_ge,
                    fill=0.0,
                    base=-128 * k,
                    pattern=[[1, SUP]],
                    channel_multiplier=-1,
                )

            # weights, transposed into [d-partition, d-chunk, m] and cast bf16.
            wqk_bf = cpool.tile([128, NDC, 2 * DL], bf16, name="wqk_bf")
            wv_bf = cpool.tile([128, NDC, DL], bf16, name="wv_bf")
            wo_bf = cpool.tile([128, NDC, DL], bf16, name="wo_bf")
            # head-permuted Wo^T for the per-head AllGather of the last chunk:
            # chunk c7 = (h, half) holds global-d rows r*192 + h*64 + w for
            # r in {2*half, 2*half+1}, matching couts7[h][128*half : ...].
            wo7_bf = cpool.tile([128, NDC, DL], bf16, name="wo7_bf")
            for (par, sb, mdim, permuted) in (
                (wqk_p, wqk_bf, 2 * DL, None),
                (wv_p, wv_bf, DL, None),
                (wo_p, wo_bf, DL, wo7_bf),
            ):
                for m0 in range(0, mdim, 128):
                    R = min(128, mdim - m0)
                    wf = wstage.tile([128, D], f32, name="wf", tag="wf")
                    nc.sync.dma_start(out=wf[:R, :], in_=par[m0 : m0 + R, :])
                    wb = wstage.tile([128, D], bf16, name="wb", tag="wb")
                    nc.vector.tensor_copy(wb[:R, :], wf[:R, :])
                    wtp = mmpsum.tile([128, NDC, 128], bf16, name="wtp", tag="mm")
                    for dc in range(NDC):
                        nc.tensor.transpose(
                            wtp[:, dc, :R],
                            wb[:R, dc * 128 : (dc + 1) * 128],
                            ident[:R, :R],
                        )
                    nc.vector.tensor_copy(sb[:, :, m0 : m0 + R], wtp[:, :, :R])
                    if permuted is not None:
                        # permute columns into (h, half)-chunk order first
                        # (transpose input APs must be flat [p, n]).
                        wb4 = wb[:R, :].rearrange(
                            "p (r hh w) -> p r hh w", r=4, hh=HL
                        )
                        wbp = wstage.tile([128, D], bf16, name="wbp", tag="wbp")
                        wbp5 = wbp[:R, :].rearrange(
                            "p (hh half r w) -> p hh half r w", hh=HL, half=2, r=2
                        )
                        for half in (0, 1):
                            nc.vector.tensor_copy(
                                wbp5[:, :, half, :, :],
                                wb4[:, 2 * half : 2 * half + 2, :, :].rearrange(
                                    "p r hh w -> p hh r w"
                                ),
                            )
                        wtp7 = mmpsum.tile(
                            [128, NDC, 128], bf16, name="wtp7", tag="mm"
                        )
                        for c7 in range(NDC):
                            nc.tensor.transpose(
                                wtp7[:, c7, :R],
                                wbp[:R, c7 * 128 : (c7 + 1) * 128],
                                ident[:R, :R],
                            )
                        nc.vector.tensor_copy(
                            permuted[:, :, m0 : m0 + R], wtp7[:, :, :R]
                        )

            bqk_sb = cpool.tile([128, 2 * DL // 128, 1], f32, name="bqk_sb")
            nc.sync.dma_start(
                out=bqk_sb[:, :, :], in_=bqk_p[:, :].rearrange("(c p) o -> p c o", p=128)
            )
            # v bias as a [1, DL] row broadcast to all 128 partitions (added
            # into v65's value columns during phase 2).
            bvrow = cpool.tile([1, DL], f32, name="bvrow")
            nc.sync.dma_start(out=bvrow[:, :], in_=bv_p[:, :].rearrange("m o -> o m"))
            bvf = cpool.tile([128, DL], f32, name="bvf")
            nc.gpsimd.partition_broadcast(bvf[:, :], bvrow[:, :], channels=128)
            bo0_sb = cpool.tile([128, 1], f32, name="bo0_sb")
            nc.sync.dma_start(out=bo0_sb[:, :], in_=bo_p[0:128, :])
            bo1_sb = cpool.tile([64, 1], f32, name="bo1_sb")
            nc.sync.dma_start(out=bo1_sb[:, :], in_=bo_p[128:DL, :])

            # ---------------- persistent activations ----------------
            # [128, 2, S]: partitions 0-63 slot0=h0 slot1=h2; 64-127 slot0=h1
            # slot1=h2 (copy). Row-tiled QK matmuls read matching halves.
            qT2 = cpool.tile([128, 2, S], bf16, name="qT2")
            kT2 = cpool.tile([128, 2, S], bf16, name="kT2")
            # v65: per sj-chunk j, per head h: cols h*65..h*65+63 = v values
            # + bv, col h*65+64 = ones (denominator -> po row 64).
            v65 = cpool.tile([128, NKC, HL * (HD + 1)], bf16, name="v65")
            poT = cpool.tile([64, HL, S], bf16, name="poT")  # preout^T per head

            nc.gpsimd.memset(v65[:, :, :], 1.0)

            # ---------------- phase 0-2: x^T, qk^T, v ----------------
            with tc.tile_pool(name="xt", bufs=1) as xtpool:
                xT = xtpool.tile([128, NDC, S], bf16, name="xT")  # 48KB/partition
                for t in range(NSUP):
                    sc = slice(t * SUP, (t + 1) * SUP)
                    for sub in range(4):
                        s0 = t * SUP + sub * 128
                        xf = xstage.tile([128, D], f32, name="xf", tag="xf")
                        nc.sync.dma_start(out=xf[:, :], in_=x_p[s0 : s0 + 128, :])
                        xb = xstage.tile([128, D], bf16, name="xb", tag="xb")
                        nc.gpsimd.tensor_copy(xb[:, :], xf[:, :])
                        tp = mmpsum.tile([128, NDC, 128], bf16, name="tp", tag="mm")
                        for dc in range(NDC):
                            nc.tensor.transpose(
                                tp[:, dc, :], xb[:, dc * 128 : (dc + 1) * 128],
                                ident[:, :],
                            )
                        nc.vector.tensor_copy(xT[:, :, s0 : s0 + 128], tp[:, :, :])

                    # qk^T for this superchunk: out [m, s]; drains routed into
                    # the row-tiled qT2/kT2 layout (h2 written to both halves).
                    for mc in range(2 * DL // 128):
                        ps = mmpsum.tile([128, 512], f32, name="ps", tag="mm")
                        for dc in range(NDC):
                            nc.tensor.matmul(
                                ps[:, :],
                                lhsT=wqk_bf[:, dc, mc * 128 : (mc + 1) * 128],
                                rhs=xT[:, dc, sc],
                                start=(dc == 0),
                                stop=(dc == NDC - 1),
                            )
                        lo, hi = ps[0:64, :], ps[64:128, :]
                        blo, bhi = bqk_sb[0:64, mc, :], bqk_sb[64:128, mc, :]
                        if mc == 0:  # q h0, q h1
                            nc.vector.tensor_scalar_add(qT2[0:64, 0, sc], lo, blo)
                            nc.vector.tensor_scalar_add(qT2[64:128, 0, sc], hi, bhi)
                        elif mc == 1:  # q h2 (dup), k h0
                            nc.vector.tensor_scalar_add(qT2[0:64, 1, sc], lo, blo)
                            nc.vector.tensor_scalar_add(qT2[64:128, 1, sc], lo, blo)
                            nc.vector.tensor_scalar_add(kT2[0:64, 0, sc], hi, bhi)
                        else:  # k h1, k h2 (dup)
                            nc.vector.tensor_scalar_add(kT2[64:128, 0, sc], lo, blo)
                            nc.vector.tensor_scalar_add(kT2[0:64, 1, sc], hi, bhi)
                            nc.vector.tensor_scalar_add(kT2[64:128, 1, sc], hi, bhi)

                    # v for this superchunk (bias folded in here)
                    for sub in range(4):
                        j = t * 4 + sub
                        pv = mmpsum.tile([128, 512], f32, name="pv", tag="mm")
                        for dc in range(NDC):
                            nc.tensor.matmul(
                                pv[:, 0:DL],
                                lhsT=xT[:, dc, j * 128 : (j + 1) * 128],
                                rhs=wv_bf[:, dc, :],
                                start=(dc == 0),
                                stop=(dc == NDC - 1),
                            )
                        nc.vector.tensor_tensor(
                            v65[:, j, :].rearrange("p (h w) -> p h w", h=HL)[
                                :, :, 0:HD
                            ],
                            pv[:, 0:DL].rearrange("p (h w) -> p h w", h=HL),
                            bvf[:, :].rearrange("p (h w) -> p h w", h=HL),
                            mybir.AluOpType.add,
                        )

            # ---------------- phase 3: flash attention (logits transposed) ----------------
            def vsl(j, h):  # v65 slice for (chunk j, head h): [128, 65]
                return v65[:, j, :].rearrange("p (hh w) -> p hh w", hh=HL)[:, h, :]

            def normalize(po, h, sc):
                rc = pspool.tile([1, 512], f32, name="rc", tag="rc")
                nc.vector.tensor_copy(rc[:, :], po[64:65, :])
                bcs = bcpool.tile([64, 512], f32, name="bcs", tag="bc")
                nc.gpsimd.partition_broadcast(bcs[:, :], rc[:, :], channels=64)
                nc.vector.reciprocal_approx_fast(out=bcs[:, :], in_=bcs[:, :])
                nc.vector.tensor_mul(poT[:, h, sc], po[0:64, :], bcs[:, :])

            for t in range(NSUP):
                si0 = t * SUP
                sc = slice(si0, si0 + SUP)
                n_j = 4 * t + 4

                # heads 0/1: one j per step, the two heads' QK matmuls run
                # concurrently on PE row-tiles (0,0) / (64,0).
                po0 = popsum.tile([65, 512], f32, name="po0", tag="po")
                po1 = popsum.tile([65, 512], f32, name="po1", tag="po")
                for j in range(n_j):
                    krel = j - 4 * t
                    off = 128 * krel if krel > 0 else 0
                    lg = lgpsum.tile([128, 2, 512], f32, name="lg", tag="lg")
                    aT = atpool.tile([128, 2, 512], bf16, name="aT", tag="at")
                    sj = slice(128 * j, 128 * (j + 1))
                    nc.tensor.matmul(
                        lg[:, 0, off:],
                        lhsT=kT2[0:64, 0, sj],
                        rhs=qT2[0:64, 0, si0 + off : si0 + SUP],
                        start=True, stop=True,
                        tile_position=(0, 0),
                    )
                    nc.tensor.matmul(
                        lg[:, 1, off:],
                        lhsT=kT2[64:128, 0, sj],
                        rhs=qT2[64:128, 0, si0 + off : si0 + SUP],
                        start=True, stop=True,
                        tile_position=(64, 0),
                    )
                    nc.scalar.activation(
                        aT[:, :, off:], lg[:, :, off:], EXP, scale=0.125
                    )
                    if krel >= 0:
                        for half in (0, 1):
                            nc.vector.tensor_mul(
                                aT[:, half, off:],
                                aT[:, half, off:],
                                masks[:, krel, off:],
                            )
                    for half, po in ((0, po0), (1, po1)):
                        nc.tensor.matmul(
                            po[:, off:],
                            lhsT=vsl(j, half),
                            rhs=aT[:, half, off:],
                            start=(j == 0),
                            stop=(j == n_j - 1),
                        )
                normalize(po0, 0, sc)
                normalize(po1, 1, sc)

                # head 2: j-parity pairs (even j on rows 0-63, odd on 64-127),
                # also concurrent via row tiling.
                po2 = popsum.tile([65, 512], f32, name="po2", tag="po")
                for pr in range(n_j // 2):
                    off = 256 if pr == 2 * t + 1 else 0
                    lg = lgpsum.tile([128, 2, 512], f32, name="lg", tag="lg")
                    aT = atpool.tile([128, 2, 512], bf16, name="aT", tag="at")
                    for half in (0, 1):
                        j = 2 * pr + half
                        sj = slice(128 * j, 128 * (j + 1))
                        p0, p1 = 64 * half, 64 * half + 64
                        nc.tensor.matmul(
                            lg[:, half, off:],
                            lhsT=kT2[p0:p1, 1, sj],
                            rhs=qT2[p0:p1, 1, si0 + off : si0 + SUP],
                            start=True, stop=True,
                            tile_position=(64 * half, 0),
                        )
                    nc.scalar.activation(
                        aT[:, :, off:], lg[:, :, off:], EXP, scale=0.125
                    )
                    for half in (0, 1):
                        j = 2 * pr + half
                        krel = j - 4 * t
                        if krel >= 0:
                            nc.vector.tensor_mul(
                                aT[:, half, off:],
                                aT[:, half, off:],
                                masks[:, krel, off:],
                            )
                        nc.tensor.matmul(
                            po2[:, off:],
                            lhsT=vsl(j, 2),
                            rhs=aT[:, half, off:],
                            start=(j == 0),
                            stop=(j == n_j - 1),
                        )
                normalize(po2, 2, sc)

                # ---------------- phase 4: chunked AllGather ----------------
                if t < NSUP - 1:
                    for h in range(HL):
                        nc.sync.dma_start(
                            out=cins[t][HD * h : HD * (h + 1), :],
                            in_=poT[:, h, sc],
                        )
                    nc.gpsimd.collective_compute(
                        "AllGather",
                        mybir.AluOpType.bypass,
                        replica_groups=GROUPS,
                        ins=[cins[t][:, :]],
                        outs=[couts[t][:, :]],
                    )
                else:
                    # per-head AllGather so the tail only waits on h2's small
                    # gather (h0/h1 fire before head 2's attention runs).
                    for h in range(HL):
                        nc.sync.dma_start(out=cins7[h][:, :], in_=poT[:, h, sc])
                        nc.gpsimd.collective_compute(
                            "AllGather",
                            mybir.AluOpType.bypass,
                            replica_groups=GROUPS,
                            ins=[cins7[h][:, :]],
                            outs=[couts7[h][:, :]],
                        )

            # ---------------- phase 5: output projection (dout-sharded) ----------------
            with tc.tile_pool(name="ccp", bufs=1) as ccpool:
                for c in range(NSUP):
                    last = c == NSUP - 1
                    strips = []
                    for dc in range(NDC):
                        strip = ccpool.tile(
                            [128, CW], bf16, name=f"ccs{c}_{dc}", tag=f"ccs{dc}", bufs=2
                        )
                        if last:
                            h, half = divmod(dc, 2)
                            src = couts7[h][half * 128 : (half + 1) * 128, :]
                        else:
                            src = couts[c][dc * 128 : (dc + 1) * 128, :]
                        nc.sync.dma_start(out=strip[:, :], in_=src)
                        strips.append(strip)
                    wsel = wo7_bf if last else wo_bf
                    for oc, M0, bo_sb in ((0, 128, bo0_sb), (1, 64, bo1_sb)):
                        pso = mmpsum.tile([128, 512], f32, name="pso", tag="mm")
                        for dc in range(NDC):
                            nc.tensor.matmul(
                                pso[0:M0, :],
                                lhsT=wsel[:, dc, oc * 128 : oc * 128 + M0],
                                rhs=strips[dc][:, :],
                                start=(dc == 0),
                                stop=(dc == NDC - 1),
                            )
                        ot = otpool.tile([128, 512], f32, name="ot", tag="ot")
                        nc.vector.tensor_scalar_add(
                            ot[0:M0, :], pso[0:M0, :], bo_sb[:, :]
                        )
                        nc.sync.dma_start(
                            out=out_p[
                                oc * 128 : oc * 128 + M0, c * SUP : (c + 1) * SUP
                            ],
                            in_=ot[0:M0, :],
                        )

    nc.finalize()
    return nc


def _get_nc():
    if "nc" not in _CACHE:
        _CACHE["nc"] = _build_nc()
    return _CACHE["nc"]


def _make_in_maps(x, Wq_w, Wq_b, Wk_w, Wk_b, Wv_w, Wv_b, Wo_w, Wo_b):
    f = np.float32
    in_maps = []
    for c in range(NCORES):
        b, hg = divmod(c, 4)
        r = slice(hg * DL, (hg + 1) * DL)
        in_maps.append(
            {
                "x": np.ascontiguousarray(x[b], dtype=f),
                "wqk": np.ascontiguousarray(
                    np.concatenate([Wq_w[r], Wk_w[r]], axis=0), dtype=f
                ),
                "bqk": np.ascontiguousarray(
                    np.concatenate([Wq_b[r], Wk_b[r]])[:, None], dtype=f
                ),
                "wv": np.ascontiguousarray(Wv_w[r], dtype=f),
                "bv": np.ascontiguousarray(Wv_b[r][:, None], dtype=f),
                "wo": np.ascontiguousarray(Wo_w[r], dtype=f),
                "bo": np.ascontiguousarray(Wo_b[r][:, None], dtype=f),
            }
        )
    return in_maps


def run_on_hw(in_maps, trace=False):
    from concourse.bass_utils import run_bass_kernel_spmd

    nc = _get_nc()
    return run_bass_kernel_spmd(nc, in_maps, core_ids=list(range(NCORES)), trace=trace)


def kernel(x, Wq_w, Wq_b, Wk_w, Wk_b, Wv_w, Wv_b, Wo_w, Wo_b):
    in_maps = _make_in_maps(
        np.asarray(x, dtype=np.float32),
        *[
            np.asarray(a, dtype=np.float32)
            for a in (Wq_w, Wq_b, Wk_w, Wk_b, Wv_w, Wv_b, Wo_w, Wo_b)
        ],
    )
    res = run_on_hw(in_maps, trace=False)
    out = np.empty((B, S, D), dtype=np.float32)
    for c in range(NCORES):
        b, hg = divmod(c, 4)
        out[b, :, hg * DL : (hg + 1) * DL] = res.results[c]["out"].T
    return out


# revision 7
# speedup vs baseline: 1.0317x; 1.0317x over previous
# Distributed Bass kernel: causal multi-head attention block on 8 TRN2 NeuronCores.
#
# Problem (hardcoded): x [2, 4096, 768] f32, 12 heads x 64 dim, causal attention,
#   out = softmax(mask(q k^T / 8)) v  projected by Wo, all nn.Linear with bias.
#
# Sharding: core c -> batch b = c // 4, head-group hg = c % 4 (3 heads each).
#   Per core: QKV for its 3 heads over the full sequence (tensor parallel on
#   heads), flash-style causal attention, chunked AllGathers of preout^T
#   (bf16) within each 4-core batch group -- pipelined behind attention --
#   then an output projection sharded over dout (each core computes its own
#   192 output columns, written transposed [192, 4096] and flipped on host).
#
# Key layout/perf choices (v1):
#   - logits computed TRANSPOSED ([sj, si]) so exp() output a^T feeds the a@v
#     matmul with no transpose; v carries a leading ones column per head so
#     the same matmul accumulates the softmax denominator into po row 0.
#   - QK^T matmuls have contraction K=64 (head dim) -> only half the PE rows.
#     Fix: PE row-tiling. qT/kT live as [128, 2, S] tiles: partitions 0-63
#     hold heads {0, 2}, partitions 64-127 hold heads {1, 2-copy}. Each pair
#     of QK matmuls issues to tile_position (0,0) and (64,0) and runs
#     CONCURRENTLY on the two halves of the PE array (~2x QK throughput).
#     Heads 0/1 pair per-j; head 2 pairs j-parity (even j lo, odd j hi).
#   - v bias folded into v65 values (softmax weights sum to 1 post-normalize,
#     so (sum a (v+b))/denom == preout + b) -- no post-softmax bias add.
#   - x^T produced via PE transpose; the 6 per-128-row-block PSUM->SBUF
#     drains are batched into one DVE copy via a [128, 6, 128] PSUM tile.

import numpy as np

B = 2
S = 4096
D = 768
HD = 64
NH = 12
NCORES = 8
HL = 3            # heads per core
DL = HL * HD      # 192: local q/k/v dims per core
SUP = 512         # si superchunk
NSUP = S // SUP   # 8
NKC = S // 128    # 32 sj chunks
NDC = D // 128    # 6 contraction chunks
GROUPS = [[0, 1, 2, 3], [4, 5, 6, 7]]
CW = SUP          # AllGather chunk width

_CACHE = {}


def _build_nc():
    import concourse.mybir as mybir
    from concourse import bacc
    from concourse.tile import TileContext
    from concourse.masks import make_identity

    f32 = mybir.dt.float32
    bf16 = mybir.dt.bfloat16
    EXP = mybir.ActivationFunctionType.Exp

    nc = bacc.Bacc(num_devices=NCORES)

    x_p = nc.declare_dram_parameter("x", [S, D], f32, isOutput=False)
    wqk_p = nc.declare_dram_parameter("wqk", [2 * DL, D], f32, isOutput=False)
    bqk_p = nc.declare_dram_parameter("bqk", [2 * DL, 1], f32, isOutput=False)
    wv_p = nc.declare_dram_parameter("wv", [DL, D], f32, isOutput=False)
    bv_p = nc.declare_dram_parameter("bv", [DL, 1], f32, isOutput=False)
    wo_p = nc.declare_dram_parameter("wo", [DL, D], f32, isOutput=False)
    bo_p = nc.declare_dram_parameter("bo", [DL, 1], f32, isOutput=False)
    out_p = nc.declare_dram_parameter("out", [DL, S], f32, isOutput=True)

    cins = [nc.dram_tensor(f"cc_in{c}", [DL, CW], bf16) for c in range(NSUP)]
    couts = [nc.dram_tensor(f"cc_out{c}", [D, CW], bf16) for c in range(NSUP)]

    with TileContext(nc) as tc:
        with (
            tc.tile_pool(name="const", bufs=1) as cpool,
            tc.tile_pool(name="wstage", bufs=2) as wstage,
            tc.tile_pool(name="xstage", bufs=3) as xstage,
            tc.tile_pool(name="at", bufs=3) as atpool,
            tc.tile_pool(name="ps", bufs=2) as pspool,
            tc.tile_pool(name="bc", bufs=2) as bcpool,
            tc.tile_pool(name="ot", bufs=2) as otpool,
            tc.tile_pool(name="mm", bufs=2, space="PSUM") as mmpsum,
            tc.tile_pool(name="lg", bufs=2, space="PSUM") as lgpsum,
            tc.tile_pool(name="po", bufs=2, space="PSUM") as popsum,
        ):
            # ---------------- constants / weights ----------------
            ident = cpool.tile([128, 128], bf16, name="ident")
            make_identity(nc, ident[:, :])

            # multiplicative causal masks for the 4 diagonal sj-chunk offsets:
            # masks[p, k, f] = 1.0 if (f - p - 128k) >= 0 else 0.0
            masks = cpool.tile([128, 4, SUP], bf16, name="masks")
            nc.gpsimd.memset(masks[:, :, :], 1.0)
            for k in range(4):
                nc.gpsimd.affine_select(
                    out=masks[:, k, :],
                    in_=masks[:, k, :],
                    compare_op=mybir.AluOpType.is_ge,
                    fill=0.0,
                    base=-128 * k,
                    pattern=[[1, SUP]],
                    channel_multiplier=-1,
                )

            # weights, transposed into [d-partition, d-chunk, m] and cast bf16.
            wqk_bf = cpool.tile([128, NDC, 2 * DL], bf16, name="wqk_bf")
            wv_bf = cpool.tile([128, NDC, DL], bf16, name="wv_bf")
            wo_bf = cpool.tile([128, NDC, DL], bf16, name="wo_bf")
            for (par, sb, mdim) in (
                (wqk_p, wqk_bf, 2 * DL),
                (wv_p, wv_bf, DL),
                (wo_p, wo_bf, DL),
            ):
                for m0 in range(0, mdim, 128):
                    R = min(128, mdim - m0)
                    wf = wstage.tile([128, D], f32, name="wf", tag="wf")
                    nc.sync.dma_start(out=wf[:R, :], in_=par[m0 : m0 + R, :])
                    wb = wstage.tile([128, D], bf16, name="wb", tag="wb")
                    nc.vector.tensor_copy(wb[:R, :], wf[:R, :])
                    wtp = mmpsum.tile([128, NDC, 128], bf16, name="wtp", tag="mm")
                    for dc in range(NDC):
                        nc.tensor.transpose(
                            wtp[:, dc, :R],
                            wb[:R, dc * 128 : (dc + 1) * 128],
                            ident[:R, :R],
                        )
                    nc.vector.tensor_copy(sb[:, :, m0 : m0 + R], wtp[:, :, :R])
            bqk_sb = cpool.tile([128, 2 * DL // 128, 1], f32, name="bqk_sb")
            nc.sync.dma_start(
                out=bqk_sb[:, :, :], in_=bqk_p[:, :].rearrange("(c p) o -> p c o", p=128)
            )
            # v bias as a [1, DL] row broadcast to all 128 partitions (added
            # into v65's value columns during phase 2).
            bvrow = cpool.tile([1, DL], f32, name="bvrow")
            nc.sync.dma_start(out=bvrow[:, :], in_=bv_p[:, :].rearrange("m o -> o m"))
            bvf = cpool.tile([128, DL], f32, name="bvf")
            nc.gpsimd.partition_broadcast(bvf[:, :], bvrow[:, :], channels=128)
            bo0_sb = cpool.tile([128, 1], f32, name="bo0_sb")
            nc.sync.dma_start(out=bo0_sb[:, :], in_=bo_p[0:128, :])
            bo1_sb = cpool.tile([64, 1], f32, name="bo1_sb")
            nc.sync.dma_start(out=bo1_sb[:, :], in_=bo_p[128:DL, :])

            # ---------------- persistent activations ----------------
            # [128, 2, S]: partitions 0-63 slot0=h0 slot1=h2; 64-127 slot0=h1
            # slot1=h2 (copy). Row-tiled QK matmuls read matching halves.
            qT2 = cpool.tile([128, 2, S], bf16, name="qT2")
            kT2 = cpool.tile([128, 2, S], bf16, name="kT2")
            # v65: per sj-chunk j, per head h: cols h*65..h*65+63 = v values
            # + bv, col h*65+64 = ones (denominator -> po row 64).
            v65 = cpool.tile([128, NKC, HL * (HD + 1)], bf16, name="v65")
            poT = cpool.tile([64, HL, S], bf16, name="poT")  # preout^T per head

            nc.gpsimd.memset(v65[:, :, :], 1.0)

            # ---------------- phase 0-2: x^T, qk^T, v ----------------
            with tc.tile_pool(name="xt", bufs=1) as xtpool:
                xT = xtpool.tile([128, NDC, S], bf16, name="xT")  # 48KB/partition
                for t in range(NSUP):
                    sc = slice(t * SUP, (t + 1) * SUP)
                    for sub in range(4):
                        s0 = t * SUP + sub * 128
                        xf = xstage.tile([128, D], f32, name="xf", tag="xf")
                        nc.sync.dma_start(out=xf[:, :], in_=x_p[s0 : s0 + 128, :])
                        xb = xstage.tile([128, D], bf16, name="xb", tag="xb")
                        nc.vector.tensor_copy(xb[:, :], xf[:, :])
                        tp = mmpsum.tile([128, NDC, 128], bf16, name="tp", tag="mm")
                        for dc in range(NDC):
                            nc.tensor.transpose(
                                tp[:, dc, :], xb[:, dc * 128 : (dc + 1) * 128],
                                ident[:, :],
                            )
                        nc.vector.tensor_copy(xT[:, :, s0 : s0 + 128], tp[:, :, :])

                    # qk^T for this superchunk: out [m, s]; drains routed into
                    # the row-tiled qT2/kT2 layout (h2 written to both halves).
                    for mc in range(2 * DL // 128):
                        ps = mmpsum.tile([128, 512], f32, name="ps", tag="mm")
                        for dc in range(NDC):
                            nc.tensor.matmul(
                                ps[:, :],
                                lhsT=wqk_bf[:, dc, mc * 128 : (mc + 1) * 128],
                                rhs=xT[:, dc, sc],
                                start=(dc == 0),
                                stop=(dc == NDC - 1),
                            )
                        lo, hi = ps[0:64, :], ps[64:128, :]
                        blo, bhi = bqk_sb[0:64, mc, :], bqk_sb[64:128, mc, :]
                        if mc == 0:  # q h0, q h1
                            nc.vector.tensor_scalar_add(qT2[0:64, 0, sc], lo, blo)
                            nc.vector.tensor_scalar_add(qT2[64:128, 0, sc], hi, bhi)
                        elif mc == 1:  # q h2 (dup), k h0
                            nc.vector.tensor_scalar_add(qT2[0:64, 1, sc], lo, blo)
                            nc.vector.tensor_scalar_add(qT2[64:128, 1, sc], lo, blo)
                            nc.vector.tensor_scalar_add(kT2[0:64, 0, sc], hi, bhi)
                        else:  # k h1, k h2 (dup)
                            nc.vector.tensor_scalar_add(kT2[64:128, 0, sc], lo, blo)
                            nc.vector.tensor_scalar_add(kT2[0:64, 1, sc], hi, bhi)
                            nc.vector.tensor_scalar_add(kT2[64:128, 1, sc], hi, bhi)

                    # v for this superchunk (bias folded in here)
                    for sub in range(4):
                        j = t * 4 + sub
                        pv = mmpsum.tile([128, 512], f32, name="pv", tag="mm")
                        for dc in range(NDC):
                            nc.tensor.matmul(
                                pv[:, 0:DL],
                                lhsT=xT[:, dc, j * 128 : (j + 1) * 128],
                                rhs=wv_bf[:, dc, :],
                                start=(dc == 0),
                                stop=(dc == NDC - 1),
                            )
                        nc.vector.tensor_tensor(
                            v65[:, j, :].rearrange("p (h w) -> p h w", h=HL)[
                                :, :, 0:HD
                            ],
                            pv[:, 0:DL].rearrange("p (h w) -> p h w", h=HL),
                            bvf[:, :].rearrange("p (h w) -> p h w", h=HL),
                            mybir.AluOpType.add,
                        )

            # ---------------- phase 3: flash attention (logits transposed) ----------------
            def vsl(j, h):  # v65 slice for (chunk j, head h): [128, 65]
                return v65[:, j, :].rearrange("p (hh w) -> p hh w", hh=HL)[:, h, :]

            def normalize(po, h, sc):
                rc = pspool.tile([1, 512], f32, name="rc", tag="rc")
                nc.vector.tensor_copy(rc[:, :], po[64:65, :])
                bcs = bcpool.tile([64, 512], f32, name="bcs", tag="bc")
                nc.gpsimd.partition_broadcast(bcs[:, :], rc[:, :], channels=64)
                nc.vector.reciprocal_approx_fast(out=bcs[:, :], in_=bcs[:, :])
                nc.vector.tensor_mul(poT[:, h, sc], po[0:64, :], bcs[:, :])

            for t in range(NSUP):
                si0 = t * SUP
                sc = slice(si0, si0 + SUP)
                n_j = 4 * t + 4

                # heads 0/1: one j per step, the two heads' QK matmuls run
                # concurrently on PE row-tiles (0,0) / (64,0).
                po0 = popsum.tile([65, 512], f32, name="po0", tag="po")
                po1 = popsum.tile([65, 512], f32, name="po1", tag="po")
                for j in range(n_j):
                    krel = j - 4 * t
                    off = 128 * krel if krel > 0 else 0
                    lg = lgpsum.tile([128, 2, 512], f32, name="lg", tag="lg")
                    aT = atpool.tile([128, 2, 512], bf16, name="aT", tag="at")
                    sj = slice(128 * j, 128 * (j + 1))
                    nc.tensor.matmul(
                        lg[:, 0, off:],
                        lhsT=kT2[0:64, 0, sj],
                        rhs=qT2[0:64, 0, si0 + off : si0 + SUP],
                        start=True, stop=True,
                        tile_position=(0, 0),
                    )
                    nc.tensor.matmul(
                        lg[:, 1, off:],
                        lhsT=kT2[64:128, 0, sj],
                        rhs=qT2[64:128, 0, si0 + off : si0 + SUP],
                        start=True, stop=True,
                        tile_position=(64, 0),
                    )
                    nc.scalar.activation(
                        aT[:, :, off:], lg[:, :, off:], EXP, scale=0.125
                    )
                    if krel >= 0:
                        for half in (0, 1):
                            nc.vector.tensor_mul(
                                aT[:, half, off:],
                                aT[:, half, off:],
                                masks[:, krel, off:],
                            )
                    for half, po in ((0, po0), (1, po1)):
                        nc.tensor.matmul(
                            po[:, off:],
                            lhsT=vsl(j, half),
                            rhs=aT[:, half, off:],
                            start=(j == 0),
                            stop=(j == n_j - 1),
                        )
                normalize(po0, 0, sc)
                normalize(po1, 1, sc)

                # head 2: j-parity pairs (even j on rows 0-63, odd on 64-127),
                # also concurrent via row tiling.
                po2 = popsum.tile([65, 512], f32, name="po2", tag="po")
                for pr in range(n_j // 2):
                    off = 256 if pr == 2 * t + 1 else 0
                    lg = lgpsum.tile([128, 2, 512], f32, name="lg", tag="lg")
                    aT = atpool.tile([128, 2, 512], bf16, name="aT", tag="at")
                    for half in (0, 1):
                        j = 2 * pr + half
                        sj = slice(128 * j, 128 * (j + 1))
                        p0, p1 = 64 * half, 64 * half + 64
                        nc.tensor.matmul(
                            lg[:, half, off:],
                            lhsT=kT2[p0:p1, 1, sj],
                            rhs=qT2[p0:p1, 1, si0 + off : si0 + SUP],
                            start=True, stop=True,
                            tile_position=(64 * half, 0),
                        )
                    nc.scalar.activation(
                        aT[:, :, off:], lg[:, :, off:], EXP, scale=0.125
                    )
                    for half in (0, 1):
                        j = 2 * pr + half
                        krel = j - 4 * t
                        if krel >= 0:
                            nc.vector.tensor_mul(
                                aT[:, half, off:],
                                aT[:, half, off:],
                                masks[:, krel, off:],
                            )
                        nc.tensor.matmul(
                            po2[:, off:],
                            lhsT=vsl(j, 2),
                            rhs=aT[:, half, off:],
                            start=(j == 0),
                            stop=(j == n_j - 1),
                        )
                normalize(po2, 2, sc)

                # ---------------- phase 4: chunked AllGather ----------------
                for h in range(HL):
                    nc.sync.dma_start(
                        out=cins[t][HD * h : HD * (h + 1), :],
                        in_=poT[:, h, sc],
                    )
                nc.gpsimd.collective_compute(
                    "AllGather",
                    mybir.AluOpType.bypass,
                    replica_groups=GROUPS,
                    ins=[cins[t][:, :]],
                    outs=[couts[t][:, :]],
                )

            # ---------------- phase 5: output projection (dout-sharded) ----------------
            with tc.tile_pool(name="ccp", bufs=1) as ccpool:
                for c in range(NSUP):
                    strips = []
                    for dc in range(NDC):
                        strip = ccpool.tile(
                            [128, CW], bf16, name=f"ccs{c}_{dc}", tag=f"ccs{dc}", bufs=2
                        )
                        nc.sync.dma_start(
                            out=strip[:, :], in_=couts[c][dc * 128 : (dc + 1) * 128, :]
                        )
                        strips.append(strip)
                    for oc, M0, bo_sb in ((0, 128, bo0_sb), (1, 64, bo1_sb)):
                        pso = mmpsum.tile([128, 512], f32, name="pso", tag="mm")
                        for dc in range(NDC):
                            nc.tensor.matmul(
                                pso[0:M0, :],
                                lhsT=wo_bf[:, dc, oc * 128 : oc * 128 + M0],
                                rhs=strips[dc][:, :],
                                start=(dc == 0),
                                stop=(dc == NDC - 1),
                            )
                        ot = otpool.tile([128, 512], f32, name="ot", tag="ot")
                        nc.vector.tensor_scalar_add(
                            ot[0:M0, :], pso[0:M0, :], bo_sb[:, :]
                        )
                        nc.sync.dma_start(
                            out=out_p[
                                oc * 128 : oc * 128 + M0, c * SUP : (c + 1) * SUP
                            ],
                            in_=ot[0:M0, :],
                        )

    nc.finalize()
    return nc


def _get_nc():
    if "nc" not in _CACHE:
        _CACHE["nc"] = _build_nc()
    return _CACHE["nc"]


def _make_in_maps(x, Wq_w, Wq_b, Wk_w, Wk_b, Wv_w, Wv_b, Wo_w, Wo_b):
    f = np.float32
    in_maps = []
    for c in range(NCORES):
        b, hg = divmod(c, 4)
        r = slice(hg * DL, (hg + 1) * DL)
        in_maps.append(
            {
                "x": np.ascontiguousarray(x[b], dtype=f),
                "wqk": np.ascontiguousarray(
                    np.concatenate([Wq_w[r], Wk_w[r]], axis=0), dtype=f
                ),
                "bqk": np.ascontiguousarray(
                    np.concatenate([Wq_b[r], Wk_b[r]])[:, None], dtype=f
                ),
                "wv": np.ascontiguousarray(Wv_w[r], dtype=f),
                "bv": np.ascontiguousarray(Wv_b[r][:, None], dtype=f),
                "wo": np.ascontiguousarray(Wo_w[r], dtype=f),
                "bo": np.ascontiguousarray(Wo_b[r][:, None], dtype=f),
            }
        )
    return in_maps


def run_on_hw(in_maps, trace=False):
    from concourse.bass_utils import run_bass_kernel_spmd

    nc = _get_nc()
    return run_bass_kernel_spmd(nc, in_maps, core_ids=list(range(NCORES)), trace=trace)


def kernel(x, Wq_w, Wq_b, Wk_w, Wk_b, Wv_w, Wv_b, Wo_w, Wo_b):
    in_maps = _make_in_maps(
        np.asarray(x, dtype=np.float32),
        *[
            np.asarray(a, dtype=np.float32)
            for a in (Wq_w, Wq_b, Wk_w, Wk_b, Wv_w, Wv_b, Wo_w, Wo_b)
        ],
    )
    res = run_on_hw(in_maps, trace=False)
    out = np.empty((B, S, D), dtype=np.float32)
    for c in range(NCORES):
        b, hg = divmod(c, 4)
        out[b, :, hg * DL : (hg + 1) * DL] = res.results[c]["out"].T
    return out


# revision 8
# speedup vs baseline: 1.0359x; 1.0041x over previous
# Distributed Bass kernel: causal multi-head attention block on 8 TRN2 NeuronCores.
#
# Problem (hardcoded): x [2, 4096, 768] f32, 12 heads x 64 dim, causal attention,
#   out = softmax(mask(q k^T / 8)) v  projected by Wo, all nn.Linear with bias.
#
# Sharding: core c -> batch b = c // 4, head-group hg = c % 4 (3 heads each).
#   Per core: QKV for its 3 heads over the full sequence (tensor parallel on
#   heads), flash-style causal attention, chunked AllGathers of preout^T
#   (bf16) within each 4-core batch group -- pipelined behind attention --
#   then an output projection sharded over dout (each core computes its own
#   192 output columns, written transposed [192, 4096] and flipped on host).
#
# Key layout/perf choices (v1):
#   - logits computed TRANSPOSED ([sj, si]) so exp() output a^T feeds the a@v
#     matmul with no transpose; v carries a leading ones column per head so
#     the same matmul accumulates the softmax denominator into po row 0.
#   - v bias folded into v65 values (softmax weights sum to 1 post-normalize,
#     so (sum a (v+b))/denom == preout + b) -- no post-softmax bias add.
#   - x^T produced via PE transpose; the 6 per-128-row-block PSUM->SBUF
#     drains are batched into one DVE copy via a [128, 6, 128] PSUM tile.

import numpy as np

B = 2
S = 4096
D = 768
HD = 64
NH = 12
NCORES = 8
HL = 3            # heads per core
DL = HL * HD      # 192: local q/k/v dims per core
SUP = 512         # si superchunk
NSUP = S // SUP   # 8
NKC = S // 128    # 32 sj chunks
NDC = D // 128    # 6 contraction chunks
GROUPS = [[0, 1, 2, 3], [4, 5, 6, 7]]
CW = SUP          # AllGather chunk width

_CACHE = {}


def _build_nc():
    import concourse.mybir as mybir
    from concourse import bacc
    from concourse.tile import TileContext
    from concourse.masks import make_identity

    f32 = mybir.dt.float32
    bf16 = mybir.dt.bfloat16
    EXP = mybir.ActivationFunctionType.Exp

    nc = bacc.Bacc(num_devices=NCORES)

    x_p = nc.declare_dram_parameter("x", [S, D], f32, isOutput=False)
    wqk_p = nc.declare_dram_parameter("wqk", [2 * DL, D], f32, isOutput=False)
    bqk_p = nc.declare_dram_parameter("bqk", [2 * DL, 1], f32, isOutput=False)
    wv_p = nc.declare_dram_parameter("wv", [DL, D], f32, isOutput=False)
    bv_p = nc.declare_dram_parameter("bv", [DL, 1], f32, isOutput=False)
    wo_p = nc.declare_dram_parameter("wo", [DL, D], f32, isOutput=False)
    bo_p = nc.declare_dram_parameter("bo", [DL, 1], f32, isOutput=False)
    out_p = nc.declare_dram_parameter("out", [DL, S], f32, isOutput=True)

    cins = [nc.dram_tensor(f"cc_in{c}", [DL, CW], bf16) for c in range(NSUP)]
    couts = [nc.dram_tensor(f"cc_out{c}", [D, CW], bf16) for c in range(NSUP)]

    with TileContext(nc) as tc:
        with (
            tc.tile_pool(name="const", bufs=1) as cpool,
            tc.tile_pool(name="wstage", bufs=2) as wstage,
            tc.tile_pool(name="xstage", bufs=3) as xstage,
            tc.tile_pool(name="at", bufs=3) as atpool,
            tc.tile_pool(name="ps", bufs=2) as pspool,
            tc.tile_pool(name="bc", bufs=2) as bcpool,
            tc.tile_pool(name="ot", bufs=2) as otpool,
            tc.tile_pool(name="mm", bufs=2, space="PSUM") as mmpsum,
            tc.tile_pool(name="lg", bufs=2, space="PSUM") as lgpsum,
            tc.tile_pool(name="po", bufs=2, space="PSUM") as popsum,
        ):
            # ---------------- constants / weights ----------------
            ident = cpool.tile([128, 128], bf16, name="ident")
            make_identity(nc, ident[:, :])

            # multiplicative causal masks for the 4 diagonal sj-chunk offsets:
            # masks[p, k, f] = 1.0 if (f - p - 128k) >= 0 else 0.0
            masks = cpool.tile([128, 4, SUP], bf16, name="masks")
            nc.gpsimd.memset(masks[:, :, :], 1.0)
            for k in range(4):
                nc.gpsimd.affine_select(
                    out=masks[:, k, :],
                    in_=masks[:, k, :],
                    compare_op=mybir.AluOpType.is_ge,
                    fill=0.0,
                    base=-128 * k,
                    pattern=[[1, SUP]],
                    channel_multiplier=-1,
                )

            # weights, transposed into [d-partition, d-chunk, m] and cast bf16.
            wqk_bf = cpool.tile([128, NDC, 2 * DL], bf16, name="wqk_bf")
            wv_bf = cpool.tile([128, NDC, DL], bf16, name="wv_bf")
            wo_bf = cpool.tile([128, NDC, DL], bf16, name="wo_bf")
            for (par, sb, mdim) in (
                (wqk_p, wqk_bf, 2 * DL),
                (wv_p, wv_bf, DL),
                (wo_p, wo_bf, DL),
            ):
                for m0 in range(0, mdim, 128):
                    R = min(128, mdim - m0)
                    wf = wstage.tile([128, D], f32, name="wf", tag="wf")
                    nc.sync.dma_start(out=wf[:R, :], in_=par[m0 : m0 + R, :])
                    wb = wstage.tile([128, D], bf16, name="wb", tag="wb")
                    nc.vector.tensor_copy(wb[:R, :], wf[:R, :])
                    wtp = mmpsum.tile([128, NDC, 128], bf16, name="wtp", tag="mm")
                    for dc in range(NDC):
                        nc.tensor.transpose(
                            wtp[:, dc, :R],
                            wb[:R, dc * 128 : (dc + 1) * 128],
                            ident[:R, :R],
                        )
                    nc.vector.tensor_copy(sb[:, :, m0 : m0 + R], wtp[:, :, :R])
            bqk_sb = cpool.tile([128, 2 * DL // 128, 1], f32, name="bqk_sb")
            nc.sync.dma_start(
                out=bqk_sb[:, :, :], in_=bqk_p[:, :].rearrange("(c p) o -> p c o", p=128)
            )
            # v bias as a [1, DL] row broadcast to all 128 partitions (added
            # into v65's value columns during phase 2).
            bvrow = cpool.tile([1, DL], f32, name="bvrow")
            nc.sync.dma_start(out=bvrow[:, :], in_=bv_p[:, :].rearrange("m o -> o m"))
            bvf = cpool.tile([128, DL], f32, name="bvf")
            nc.gpsimd.partition_broadcast(bvf[:, :], bvrow[:, :], channels=128)
            bo0_sb = cpool.tile([128, 1], f32, name="bo0_sb")
            nc.sync.dma_start(out=bo0_sb[:, :], in_=bo_p[0:128, :])
            bo1_sb = cpool.tile([64, 1], f32, name="bo1_sb")
            nc.sync.dma_start(out=bo1_sb[:, :], in_=bo_p[128:DL, :])

            # ---------------- persistent activations ----------------
            qT = cpool.tile([64, HL, S], bf16, name="qT")  # [64, 3, 4096]
            kT = cpool.tile([64, HL, S], bf16, name="kT")
            # v65: per sj-chunk j, per head h: cols h*65..h*65+63 = v values
            # + bv, col h*65+64 = ones (denominator -> po row 64).
            v65 = cpool.tile([128, NKC, HL * (HD + 1)], bf16, name="v65")
            poT = cpool.tile([64, HL, S], bf16, name="poT")  # preout^T per head

            nc.gpsimd.memset(v65[:, :, :], 1.0)

            # ---------------- phase 0-2: x^T, qk^T, v ----------------
            with tc.tile_pool(name="xt", bufs=1) as xtpool:
                xT = xtpool.tile([128, NDC, S], bf16, name="xT")  # 48KB/partition
                for t in range(NSUP):
                    sc = slice(t * SUP, (t + 1) * SUP)
                    for sub in range(4):
                        s0 = t * SUP + sub * 128
                        xf = xstage.tile([128, D], f32, name="xf", tag="xf")
                        nc.sync.dma_start(out=xf[:, :], in_=x_p[s0 : s0 + 128, :])
                        xb = xstage.tile([128, D], bf16, name="xb", tag="xb")
                        nc.vector.tensor_copy(xb[:, :], xf[:, :])
                        tp = mmpsum.tile([128, NDC, 128], bf16, name="tp", tag="mm")
                        for dc in range(NDC):
                            nc.tensor.transpose(
                                tp[:, dc, :], xb[:, dc * 128 : (dc + 1) * 128],
                                ident[:, :],
                            )
                        nc.vector.tensor_copy(xT[:, :, s0 : s0 + 128], tp[:, :, :])

                    # qk^T for this superchunk: out [m, s]; drains routed into
                    # the row-tiled qT2/kT2 layout (h2 written to both halves).
                    for mc in range(2 * DL // 128):
                        ps = mmpsum.tile([128, 512], f32, name="ps", tag="mm")
                        for dc in range(NDC):
                            nc.tensor.matmul(
                                ps[:, :],
                                lhsT=wqk_bf[:, dc, mc * 128 : (mc + 1) * 128],
                                rhs=xT[:, dc, sc],
                                start=(dc == 0),
                                stop=(dc == NDC - 1),
                            )
                        for half in (0, 1):
                            g = mc * 128 + half * 64  # global row in [q(192); k(192)]
                            dst = (
                                qT[:, g // 64, sc]
                                if g < DL
                                else kT[:, (g - DL) // 64, sc]
                            )
                            nc.vector.tensor_scalar_add(
                                dst,
                                ps[half * 64 : half * 64 + 64, :],
                                bqk_sb[half * 64 : half * 64 + 64, mc, :],
                            )

                    # v for this superchunk (bias folded in here)
                    for sub in range(4):
                        j = t * 4 + sub
                        pv = mmpsum.tile([128, 512], f32, name="pv", tag="mm")
                        for dc in range(NDC):
                            nc.tensor.matmul(
                                pv[:, 0:DL],
                                lhsT=xT[:, dc, j * 128 : (j + 1) * 128],
                                rhs=wv_bf[:, dc, :],
                                start=(dc == 0),
                                stop=(dc == NDC - 1),
                            )
                        nc.vector.tensor_tensor(
                            v65[:, j, :].rearrange("p (h w) -> p h w", h=HL)[
                                :, :, 0:HD
                            ],
                            pv[:, 0:DL].rearrange("p (h w) -> p h w", h=HL),
                            bvf[:, :].rearrange("p (h w) -> p h w", h=HL),
                            mybir.AluOpType.add,
                        )

            # ---------------- phase 3: flash attention (logits transposed) ----------------
            def vsl(j, h):  # v65 slice for (chunk j, head h): [128, 65]
                return v65[:, j, :].rearrange("p (hh w) -> p hh w", hh=HL)[:, h, :]

            def normalize(po, h, sc):
                rc = pspool.tile([1, 512], f32, name="rc", tag="rc")
                nc.vector.tensor_copy(rc[:, :], po[64:65, :])
                bcs = bcpool.tile([64, 512], f32, name="bcs", tag="bc")
                nc.gpsimd.partition_broadcast(bcs[:, :], rc[:, :], channels=64)
                nc.vector.reciprocal_approx_fast(out=bcs[:, :], in_=bcs[:, :])
                nc.vector.tensor_mul(poT[:, h, sc], po[0:64, :], bcs[:, :])

            for t in range(NSUP):
                si0 = t * SUP
                sc = slice(si0, si0 + SUP)
                n_j = 4 * t + 4
                for h in range(HL):
                    po = popsum.tile([65, 512], f32, name="po", tag="po")
                    for pr in range(n_j // 2):
                        off = 256 if pr == 2 * t + 1 else 0
                        lg = lgpsum.tile([128, 2, 512], f32, name="lg", tag="lg")
                        aT = atpool.tile([128, 2, 512], bf16, name="aT", tag="at")
                        for half in (0, 1):
                            j = 2 * pr + half
                            sj = slice(128 * j, 128 * (j + 1))
                            nc.tensor.matmul(
                                lg[:, half, off:],
                                lhsT=kT[:, h, sj],
                                rhs=qT[:, h, si0 + off : si0 + SUP],
                                start=True,
                                stop=True,
                            )
                        nc.scalar.activation(
                            aT[:, :, off:], lg[:, :, off:], EXP, scale=0.125
                        )
                        for half in (0, 1):
                            j = 2 * pr + half
                            krel = j - 4 * t
                            if krel >= 0:
                                nc.vector.tensor_mul(
                                    aT[:, half, off:],
                                    aT[:, half, off:],
                                    masks[:, krel, off:],
                                )
                            nc.tensor.matmul(
                                po[:, off:],
                                lhsT=vsl(j, h),
                                rhs=aT[:, half, off:],
                                start=(j == 0),
                                stop=(j == n_j - 1),
                            )
                    normalize(po, h, sc)
                # ---------------- phase 4: chunked AllGather ----------------
                for h in range(HL):
                    nc.sync.dma_start(
                        out=cins[t][HD * h : HD * (h + 1), :],
                        in_=poT[:, h, sc],
                    )
                nc.gpsimd.collective_compute(
                    "AllGather",
                    mybir.AluOpType.bypass,
                    replica_groups=GROUPS,
                    ins=[cins[t][:, :]],
                    outs=[couts[t][:, :]],
                )

            # ---------------- phase 5: output projection (dout-sharded) ----------------
            with tc.tile_pool(name="ccp", bufs=1) as ccpool:
                for c in range(NSUP):
                    strips = []
                    for dc in range(NDC):
                        strip = ccpool.tile(
                            [128, CW], bf16, name=f"ccs{c}_{dc}", tag=f"ccs{dc}", bufs=2
                        )
                        nc.sync.dma_start(
                            out=strip[:, :], in_=couts[c][dc * 128 : (dc + 1) * 128, :]
                        )
                        strips.append(strip)
                    for oc, M0, bo_sb in ((0, 128, bo0_sb), (1, 64, bo1_sb)):
                        pso = mmpsum.tile([128, 512], f32, name="pso", tag="mm")
                        for dc in range(NDC):
                            nc.tensor.matmul(
                                pso[0:M0, :],
                                lhsT=wo_bf[:, dc, oc * 128 : oc * 128 + M0],
                                rhs=strips[dc][:, :],
                                start=(dc == 0),
                                stop=(dc == NDC - 1),
                            )
                        ot = otpool.tile([128, 512], f32, name="ot", tag="ot")
                        nc.vector.tensor_scalar_add(
                            ot[0:M0, :], pso[0:M0, :], bo_sb[:, :]
                        )
                        nc.sync.dma_start(
                            out=out_p[
                                oc * 128 : oc * 128 + M0, c * SUP : (c + 1) * SUP
                            ],
                            in_=ot[0:M0, :],
                        )

    nc.finalize()
    return nc


def _get_nc():
    if "nc" not in _CACHE:
        _CACHE["nc"] = _build_nc()
    return _CACHE["nc"]


def _make_in_maps(x, Wq_w, Wq_b, Wk_w, Wk_b, Wv_w, Wv_b, Wo_w, Wo_b):
    f = np.float32
    in_maps = []
    for c in range(NCORES):
        b, hg = divmod(c, 4)
        r = slice(hg * DL, (hg + 1) * DL)
        in_maps.append(
            {
                "x": np.ascontiguousarray(x[b], dtype=f),
                "wqk": np.ascontiguousarray(
                    np.concatenate([Wq_w[r], Wk_w[r]], axis=0), dtype=f
                ),
                "bqk": np.ascontiguousarray(
                    np.concatenate([Wq_b[r], Wk_b[r]])[:, None], dtype=f
                ),
                "wv": np.ascontiguousarray(Wv_w[r], dtype=f),
                "bv": np.ascontiguousarray(Wv_b[r][:, None], dtype=f),
                "wo": np.ascontiguousarray(Wo_w[r], dtype=f),
                "bo": np.ascontiguousarray(Wo_b[r][:, None], dtype=f),
            }
        )
    return in_maps


def run_on_hw(in_maps, trace=False):
    from concourse.bass_utils import run_bass_kernel_spmd

    nc = _get_nc()
    return run_bass_kernel_spmd(nc, in_maps, core_ids=list(range(NCORES)), trace=trace)


def kernel(x, Wq_w, Wq_b, Wk_w, Wk_b, Wv_w, Wv_b, Wo_w, Wo_b):
    in_maps = _make_in_maps(
        np.asarray(x, dtype=np.float32),
        *[
            np.asarray(a, dtype=np.float32)
            for a in (Wq_w, Wq_b, Wk_w, Wk_b, Wv_w, Wv_b, Wo_w, Wo_b)
        ],
    )
    res = run_on_hw(in_maps, trace=False)
    out = np.empty((B, S, D), dtype=np.float32)
    for c in range(NCORES):
        b, hg = divmod(c, 4)
        out[b, :, hg * DL : (hg + 1) * DL] = res.results[c]["out"].T
    return out


# revision 9
# speedup vs baseline: 1.0740x; 1.0368x over previous
# Distributed Bass kernel: causal multi-head attention block on 8 TRN2 NeuronCores.
#
# Problem (hardcoded): x [2, 4096, 768] f32, 12 heads x 64 dim, causal attention,
#   out = softmax(mask(q k^T / 8)) v  projected by Wo, all nn.Linear with bias.
#
# Sharding: core c -> batch b = c // 4, head-group hg = c % 4 (3 heads each).
#   Per core: QKV for its 3 heads over the full sequence (tensor parallel on
#   heads), flash-style causal attention, chunked AllGathers of preout^T
#   (bf16) within each 4-core batch group -- pipelined behind attention --
#   then an output projection sharded over dout (each core computes its own
#   192 output columns, written transposed [192, 4096] and flipped on host).
#
# Key layout/perf choices (v1):
#   - logits computed TRANSPOSED ([sj, si]) so exp() output a^T feeds the a@v
#     matmul with no transpose; v carries a leading ones column per head so
#     the same matmul accumulates the softmax denominator into po row 0.
#   - v bias folded into v65 values (softmax weights sum to 1 post-normalize,
#     so (sum a (v+b))/denom == preout + b) -- no post-softmax bias add.
#   - x^T produced via PE transpose; the 6 per-128-row-block PSUM->SBUF
#     drains are batched into one DVE copy via a [128, 6, 128] PSUM tile.

import numpy as np

B = 2
S = 4096
D = 768
HD = 64
NH = 12
NCORES = 8
HL = 3            # heads per core
DL = HL * HD      # 192: local q/k/v dims per core
SUP = 512         # si superchunk
NSUP = S // SUP   # 8
NKC = S // 128    # 32 sj chunks
NDC = D // 128    # 6 contraction chunks
GROUPS = [[0, 1, 2, 3], [4, 5, 6, 7]]
CW = SUP          # AllGather chunk width

_CACHE = {}


def _build_nc():
    import concourse.mybir as mybir
    from concourse import bacc
    from concourse.tile import TileContext
    from concourse.masks import make_identity

    f32 = mybir.dt.float32
    bf16 = mybir.dt.bfloat16
    EXP = mybir.ActivationFunctionType.Exp
    IDENT = mybir.ActivationFunctionType.Identity

    nc = bacc.Bacc(num_devices=NCORES)

    x_p = nc.declare_dram_parameter("x", [S, D], f32, isOutput=False)
    wqk_p = nc.declare_dram_parameter("wqk", [2 * DL, D], f32, isOutput=False)
    bqk_p = nc.declare_dram_parameter("bqk", [2 * DL, 1], f32, isOutput=False)
    wv_p = nc.declare_dram_parameter("wv", [DL, D], f32, isOutput=False)
    bv_p = nc.declare_dram_parameter("bv", [DL, 1], f32, isOutput=False)
    wo_p = nc.declare_dram_parameter("wo", [DL, D], f32, isOutput=False)
    bo_p = nc.declare_dram_parameter("bo", [DL, 1], f32, isOutput=False)
    out_p = nc.declare_dram_parameter("out", [DL, S], f32, isOutput=True)

    cins = [nc.dram_tensor(f"cc_in{c}", [DL, CW], bf16) for c in range(NSUP)]
    couts = [nc.dram_tensor(f"cc_out{c}", [D, CW], bf16) for c in range(NSUP)]

    with TileContext(nc) as tc:
        with (
            tc.tile_pool(name="const", bufs=1) as cpool,
            tc.tile_pool(name="wstage", bufs=2) as wstage,
            tc.tile_pool(name="xstage", bufs=3) as xstage,
            tc.tile_pool(name="at", bufs=4) as atpool,
            tc.tile_pool(name="ps", bufs=2) as pspool,
            tc.tile_pool(name="bc", bufs=2) as bcpool,
            tc.tile_pool(name="ot", bufs=2) as otpool,
            tc.tile_pool(name="mm", bufs=2, space="PSUM") as mmpsum,
            tc.tile_pool(name="lg", bufs=2, space="PSUM") as lgpsum,
            tc.tile_pool(name="po", bufs=2, space="PSUM") as popsum,
        ):
            # ---------------- constants / weights ----------------
            ident = cpool.tile([128, 128], bf16, name="ident")
            make_identity(nc, ident[:, :])

            # multiplicative causal masks for the 4 diagonal sj-chunk offsets:
            # masks[p, k, f] = 1.0 if (f - p - 128k) >= 0 else 0.0
            masks = cpool.tile([128, 4, SUP], bf16, name="masks")
            nc.gpsimd.memset(masks[:, :, :], 1.0)
            for k in range(4):
                nc.gpsimd.affine_select(
                    out=masks[:, k, :],
                    in_=masks[:, k, :],
                    compare_op=mybir.AluOpType.is_ge,
                    fill=0.0,
                    base=-128 * k,
                    pattern=[[1, SUP]],
                    channel_multiplier=-1,
                )

            # weights, transposed into [d-partition, d-chunk, m] and cast bf16.
            wqk_bf = cpool.tile([128, NDC, 2 * DL], bf16, name="wqk_bf")
            wv_bf = cpool.tile([128, NDC, DL], bf16, name="wv_bf")
            wo_bf = cpool.tile([128, NDC, DL], bf16, name="wo_bf")
            # weight transposes go through the lg PSUM pool (idle during the
            # ramp) so they don't serialize with x transposes in the mm ring.
            def build_w(par, sb, mdim):
                for m0 in range(0, mdim, 128):
                    R = min(128, mdim - m0)
                    wf = wstage.tile([128, D], f32, name="wf", tag="wf")
                    nc.sync.dma_start(out=wf[:R, :], in_=par[m0 : m0 + R, :])
                    wb = wstage.tile([128, D], bf16, name="wb", tag="wb")
                    nc.vector.tensor_copy(wb[:R, :], wf[:R, :])
                    wtp = lgpsum.tile([128, NDC, 128], bf16, name="wtp", tag="lg")
                    for dc in range(NDC):
                        nc.tensor.transpose(
                            wtp[:, dc, :R],
                            wb[:R, dc * 128 : (dc + 1) * 128],
                            ident[:R, :R],
                        )
                    nc.vector.tensor_copy(sb[:, :, m0 : m0 + R], wtp[:, :, :R])

            build_w(wqk_p, wqk_bf, 2 * DL)
            build_w(wv_p, wv_bf, DL)
            bqk_sb = cpool.tile([128, 2 * DL // 128, 1], f32, name="bqk_sb")
            nc.sync.dma_start(
                out=bqk_sb[:, :, :], in_=bqk_p[:, :].rearrange("(c p) o -> p c o", p=128)
            )
            # v bias as a [1, DL] row broadcast to all 128 partitions (added
            # into v65's value columns during phase 2).
            bvrow = cpool.tile([1, DL], f32, name="bvrow")
            nc.sync.dma_start(out=bvrow[:, :], in_=bv_p[:, :].rearrange("m o -> o m"))
            bvf = cpool.tile([128, DL], f32, name="bvf")
            nc.gpsimd.partition_broadcast(bvf[:, :], bvrow[:, :], channels=128)
            bo0_sb = cpool.tile([128, 1], f32, name="bo0_sb")
            nc.sync.dma_start(out=bo0_sb[:, :], in_=bo_p[0:128, :])
            bo1_sb = cpool.tile([64, 1], f32, name="bo1_sb")
            nc.sync.dma_start(out=bo1_sb[:, :], in_=bo_p[128:DL, :])

            # ---------------- persistent activations ----------------
            qT = cpool.tile([64, HL, S], bf16, name="qT")  # [64, 3, 4096]
            kT = cpool.tile([64, HL, S], bf16, name="kT")
            # v65: per sj-chunk j, per head h: cols h*65..h*65+63 = v values
            # + bv, col h*65+64 = ones (denominator -> po row 64).
            v65 = cpool.tile([128, NKC, HL * (HD + 1)], bf16, name="v65")
            poT = cpool.tile([64, HL, S], bf16, name="poT")  # preout^T per head

            nc.gpsimd.memset(v65[:, :, :], 1.0)

            # ---------------- phase 0-2: x^T, qk^T, v ----------------
            with tc.tile_pool(name="xt", bufs=1) as xtpool:
                xT = xtpool.tile([128, NDC, S], bf16, name="xT")  # 48KB/partition
                for t in range(NSUP):
                    sc = slice(t * SUP, (t + 1) * SUP)
                    for sub in range(4):
                        s0 = t * SUP + sub * 128
                        xf = xstage.tile([128, D], f32, name="xf", tag="xf")
                        nc.sync.dma_start(out=xf[:, :], in_=x_p[s0 : s0 + 128, :])
                        xb = xstage.tile([128, D], bf16, name="xb", tag="xb")
                        nc.vector.tensor_copy(xb[:, :], xf[:, :])
                        tp = mmpsum.tile([128, NDC, 128], bf16, name="tp", tag="mm")
                        for dc in range(NDC):
                            nc.tensor.transpose(
                                tp[:, dc, :], xb[:, dc * 128 : (dc + 1) * 128],
                                ident[:, :],
                            )
                        nc.vector.tensor_copy(xT[:, :, s0 : s0 + 128], tp[:, :, :])

                    # qk^T for this superchunk: out [m, s]; drains routed into
                    # the row-tiled qT2/kT2 layout (h2 written to both halves).
                    for mc in range(2 * DL // 128):
                        ps = mmpsum.tile([128, 512], f32, name="ps", tag="mm")
                        for dc in range(NDC):
                            nc.tensor.matmul(
                                ps[:, :],
                                lhsT=wqk_bf[:, dc, mc * 128 : (mc + 1) * 128],
                                rhs=xT[:, dc, sc],
                                start=(dc == 0),
                                stop=(dc == NDC - 1),
                            )
                        for half in (0, 1):
                            g = mc * 128 + half * 64  # global row in [q(192); k(192)]
                            dst = (
                                qT[:, g // 64, sc]
                                if g < DL
                                else kT[:, (g - DL) // 64, sc]
                            )
                            # drain on ScalarE (idle in phase A): copy+bias+cast
                            nc.scalar.activation(
                                dst,
                                ps[half * 64 : half * 64 + 64, :],
                                IDENT,
                                bias=bqk_sb[half * 64 : half * 64 + 64, mc, :],
                            )

                    # v for this superchunk (bias folded in here); two sj
                    # chunks share one PSUM tile to halve mm-ring traffic.
                    for sp in range(2):
                        pv = mmpsum.tile([128, 2, DL], f32, name="pv", tag="mm")
                        for i in range(2):
                            j = t * 4 + sp * 2 + i
                            for dc in range(NDC):
                                nc.tensor.matmul(
                                    pv[:, i, :],
                                    lhsT=xT[:, dc, j * 128 : (j + 1) * 128],
                                    rhs=wv_bf[:, dc, :],
                                    start=(dc == 0),
                                    stop=(dc == NDC - 1),
                                )
                        for i in range(2):
                            j = t * 4 + sp * 2 + i
                            nc.vector.tensor_tensor(
                                v65[:, j, :].rearrange("p (h w) -> p h w", h=HL)[
                                    :, :, 0:HD
                                ],
                                pv[:, i, :].rearrange("p (h w) -> p h w", h=HL),
                                bvf[:, :].rearrange("p (h w) -> p h w", h=HL),
                                mybir.AluOpType.add,
                            )

            build_w(wo_p, wo_bf, DL)  # deferred: only needed in phase 5

            # ---------------- phase 3: flash attention (logits transposed) ----------------
            def vsl(j, h):  # v65 slice for (chunk j, head h): [128, 65]
                return v65[:, j, :].rearrange("p (hh w) -> p hh w", hh=HL)[:, h, :]

            def normalize(po, h, sc):
                rc = pspool.tile([1, 512], f32, name="rc", tag="rc")
                nc.vector.tensor_copy(rc[:, :], po[64:65, :])
                bcs = bcpool.tile([64, 512], f32, name="bcs", tag="bc")
                nc.gpsimd.partition_broadcast(bcs[:, :], rc[:, :], channels=64)
                nc.vector.reciprocal_approx_fast(out=bcs[:, :], in_=bcs[:, :])
                nc.vector.tensor_mul(poT[:, h, sc], po[0:64, :], bcs[:, :])

            for t in range(NSUP):
                si0 = t * SUP
                sc = slice(si0, si0 + SUP)
                n_j = 4 * t + 4
                for h in range(HL):
                    po = popsum.tile([65, 512], f32, name="po", tag="po")
                    for pr in range(n_j // 2):
                        off = 256 if pr == 2 * t + 1 else 0
                        lg = lgpsum.tile([128, 2, 512], f32, name="lg", tag="lg")
                        aT = atpool.tile([128, 2, 512], bf16, name="aT", tag="at")
                        for half in (0, 1):
                            j = 2 * pr + half
                            sj = slice(128 * j, 128 * (j + 1))
                            nc.tensor.matmul(
                                lg[:, half, off:],
                                lhsT=kT[:, h, sj],
                                rhs=qT[:, h, si0 + off : si0 + SUP],
                                start=True,
                                stop=True,
                            )
                        nc.scalar.activation(
                            aT[:, :, off:], lg[:, :, off:], EXP, scale=0.125
                        )
                        for half in (0, 1):
                            j = 2 * pr + half
                            krel = j - 4 * t
                            if krel >= 0:
                                nc.vector.tensor_mul(
                                    aT[:, half, off:],
                                    aT[:, half, off:],
                                    masks[:, krel, off:],
                                )
                            nc.tensor.matmul(
                                po[:, off:],
                                lhsT=vsl(j, h),
                                rhs=aT[:, half, off:],
                                start=(j == 0),
                                stop=(j == n_j - 1),
                            )
                    normalize(po, h, sc)
                # ---------------- phase 4: chunked AllGather ----------------
                for h in range(HL):
                    nc.sync.dma_start(
                        out=cins[t][HD * h : HD * (h + 1), :],
                        in_=poT[:, h, sc],
                    )
                nc.gpsimd.collective_compute(
                    "AllGather",
                    mybir.AluOpType.bypass,
                    replica_groups=GROUPS,
                    ins=[cins[t][:, :]],
                    outs=[couts[t][:, :]],
                )

            # ---------------- phase 5: output projection (dout-sharded) ----------------
            with tc.tile_pool(name="ccp", bufs=1) as ccpool:
                for c in range(NSUP):
                    strips = []
                    for dc in range(NDC):
                        strip = ccpool.tile(
                            [128, CW], bf16, name=f"ccs{c}_{dc}", tag=f"ccs{dc}", bufs=2
                        )
                        nc.sync.dma_start(
                            out=strip[:, :], in_=couts[c][dc * 128 : (dc + 1) * 128, :]
                        )
                        strips.append(strip)
                    for oc, M0, bo_sb in ((0, 128, bo0_sb), (1, 64, bo1_sb)):
                        pso = mmpsum.tile([128, 512], f32, name="pso", tag="mm")
                        for dc in range(NDC):
                            nc.tensor.matmul(
                                pso[0:M0, :],
                                lhsT=wo_bf[:, dc, oc * 128 : oc * 128 + M0],
                                rhs=strips[dc][:, :],
                                start=(dc == 0),
                                stop=(dc == NDC - 1),
                            )
                        ot = otpool.tile([128, 512], f32, name="ot", tag="ot")
                        nc.vector.tensor_scalar_add(
                            ot[0:M0, :], pso[0:M0, :], bo_sb[:, :]
                        )
                        nc.sync.dma_start(
                            out=out_p[
                                oc * 128 : oc * 128 + M0, c * SUP : (c + 1) * SUP
                            ],
                            in_=ot[0:M0, :],
                        )

    nc.finalize()
    return nc


def _get_nc():
    if "nc" not in _CACHE:
        _CACHE["nc"] = _build_nc()
    return _CACHE["nc"]


def _make_in_maps(x, Wq_w, Wq_b, Wk_w, Wk_b, Wv_w, Wv_b, Wo_w, Wo_b):
    f = np.float32
    in_maps = []
    for c in range(NCORES):
        b, hg = divmod(c, 4)
        r = slice(hg * DL, (hg + 1) * DL)
        in_maps.append(
            {
                "x": np.ascontiguousarray(x[b], dtype=f),
                "wqk": np.ascontiguousarray(
                    np.concatenate([Wq_w[r], Wk_w[r]], axis=0), dtype=f
                ),
                "bqk": np.ascontiguousarray(
                    np.concatenate([Wq_b[r], Wk_b[r]])[:, None], dtype=f
                ),
                "wv": np.ascontiguousarray(Wv_w[r], dtype=f),
                "bv": np.ascontiguousarray(Wv_b[r][:, None], dtype=f),
                "wo": np.ascontiguousarray(Wo_w[r], dtype=f),
                "bo": np.ascontiguousarray(Wo_b[r][:, None], dtype=f),
            }
        )
    return in_maps


def run_on_hw(in_maps, trace=False):
    from concourse.bass_utils import run_bass_kernel_spmd

    nc = _get_nc()
    return run_bass_kernel_spmd(nc, in_maps, core_ids=list(range(NCORES)), trace=trace)


def kernel(x, Wq_w, Wq_b, Wk_w, Wk_b, Wv_w, Wv_b, Wo_w, Wo_b):
    in_maps = _make_in_maps(
        np.asarray(x, dtype=np.float32),
        *[
            np.asarray(a, dtype=np.float32)
            for a in (Wq_w, Wq_b, Wk_w, Wk_b, Wv_w, Wv_b, Wo_w, Wo_b)
        ],
    )
    res = run_on_hw(in_maps, trace=False)
    out = np.empty((B, S, D), dtype=np.float32)
    for c in range(NCORES):
        b, hg = divmod(c, 4)
        out[b, :, hg * DL : (hg + 1) * DL] = res.results[c]["out"].T
    return out


# revision 16
# speedup vs baseline: 1.1657x; 1.0854x over previous
# Distributed Bass kernel: causal multi-head attention block on 8 TRN2 NeuronCores.
#
# Problem (hardcoded): x [2, 4096, 768] f32, 12 heads x 64 dim, causal attention,
#   out = softmax(mask(q k^T / 8)) v  projected by Wo, all nn.Linear with bias.
#
# Sharding: core c -> batch b = c // 4, head-group hg = c % 4 (3 heads each).
#   Per core: QKV for its 3 heads over the full sequence (tensor parallel on
#   heads), flash-style causal attention, chunked AllGathers of preout^T
#   (bf16) within each 4-core batch group -- pipelined behind attention --
#   then an output projection sharded over dout (each core computes its own
#   192 output columns, written transposed [192, 4096] and flipped on host).
#
# Key layout/perf choices (v1):
#   - logits computed TRANSPOSED ([sj, si]) so exp() output a^T feeds the a@v
#     matmul with no transpose; v carries a leading ones column per head so
#     the same matmul accumulates the softmax denominator into po row 0.
#   - v bias folded into v65 values (softmax weights sum to 1 post-normalize,
#     so (sum a (v+b))/denom == preout + b) -- no post-softmax bias add.
#   - x^T produced via PE transpose; the 6 per-128-row-block PSUM->SBUF
#     drains are batched into one DVE copy via a [128, 6, 128] PSUM tile.

import numpy as np

B = 2
S = 4096
D = 768
HD = 64
NH = 12
NCORES = 8
HL = 3            # heads per core
DL = HL * HD      # 192: local q/k/v dims per core
SUP = 512         # si superchunk
NSUP = S // SUP   # 8
NKC = S // 128    # 32 sj chunks
NDC = D // 128    # 6 contraction chunks
GROUPS = [[0, 1, 2, 3], [4, 5, 6, 7]]
CW = SUP          # AllGather chunk width

_CACHE = {}


def _build_nc():
    import concourse.mybir as mybir
    from concourse import bacc
    from concourse.tile import TileContext
    from concourse.masks import make_identity

    f32 = mybir.dt.float32
    bf16 = mybir.dt.bfloat16
    EXP = mybir.ActivationFunctionType.Exp
    IDENT = mybir.ActivationFunctionType.Identity

    nc = bacc.Bacc(num_devices=NCORES)

    x_p = nc.declare_dram_parameter("x", [S, D], f32, isOutput=False)
    wqk_p = nc.declare_dram_parameter("wqk", [2 * DL, D], f32, isOutput=False)
    bqk_p = nc.declare_dram_parameter("bqk", [2 * DL, 1], f32, isOutput=False)
    wv_p = nc.declare_dram_parameter("wv", [DL, D], f32, isOutput=False)
    bv_p = nc.declare_dram_parameter("bv", [DL, 1], f32, isOutput=False)
    wo_p = nc.declare_dram_parameter("wo", [DL, D], f32, isOutput=False)
    bo_p = nc.declare_dram_parameter("bo", [DL, 1], f32, isOutput=False)
    out_p = nc.declare_dram_parameter("out", [DL, S], f32, isOutput=True)

    cins = [nc.dram_tensor(f"cc_in{c}", [DL, CW], bf16) for c in range(NSUP)]
    couts = [nc.dram_tensor(f"cc_out{c}", [D, CW], bf16) for c in range(NSUP)]

    with TileContext(nc) as tc:
        with (
            tc.tile_pool(name="const", bufs=1) as cpool,
            tc.tile_pool(name="wstage", bufs=2) as wstage,
            tc.tile_pool(name="xstage", bufs=4) as xstage,
            tc.tile_pool(name="at", bufs=4) as atpool,
            tc.tile_pool(name="ps", bufs=2) as pspool,
            tc.tile_pool(name="bc", bufs=2) as bcpool,
            tc.tile_pool(name="ot", bufs=2) as otpool,
            tc.tile_pool(name="mm", bufs=2, space="PSUM") as mmpsum,
            tc.tile_pool(name="lg", bufs=2, space="PSUM") as lgpsum,
            tc.tile_pool(name="po", bufs=2, space="PSUM") as popsum,
        ):
            # ---------------- constants / weights ----------------
            ident = cpool.tile([128, 128], bf16, name="ident")
            make_identity(nc, ident[:, :])

            # multiplicative causal masks for the 4 diagonal sj-chunk offsets:
            # masks[p, k, f] = 1.0 if (f - p - 128k) >= 0 else 0.0
            masks = cpool.tile([128, 4, SUP], bf16, name="masks")
            nc.gpsimd.memset(masks[:, :, :], 1.0)
            for k in range(4):
                nc.gpsimd.affine_select(
                    out=masks[:, k, :],
                    in_=masks[:, k, :],
                    compare_op=mybir.AluOpType.is_ge,
                    fill=0.0,
                    base=-128 * k,
                    pattern=[[1, SUP]],
                    channel_multiplier=-1,
                )

            # weights, transposed into [d-partition, d-chunk, m] and cast bf16.
            wqk_bf = cpool.tile([128, NDC, 2 * DL], bf16, name="wqk_bf")
            wv_bf = cpool.tile([128, NDC, DL], bf16, name="wv_bf")
            wo_bf = cpool.tile([128, NDC, DL], bf16, name="wo_bf")
            # weight transposes go through the lg PSUM pool (idle during the
            # ramp) so they don't serialize with x transposes in the mm ring.
            def build_w(par, sb, mdim):
                for m0 in range(0, mdim, 128):
                    R = min(128, mdim - m0)
                    wf = wstage.tile([128, D], f32, name="wf", tag="wf")
                    nc.sync.dma_start(out=wf[:R, :], in_=par[m0 : m0 + R, :])
                    wb = wstage.tile([128, D], bf16, name="wb", tag="wb")
                    nc.vector.tensor_copy(wb[:R, :], wf[:R, :])
                    wtp = lgpsum.tile([128, NDC, 128], bf16, name="wtp", tag="lg")
                    for dc in range(NDC):
                        nc.tensor.transpose(
                            wtp[:, dc, :R],
                            wb[:R, dc * 128 : (dc + 1) * 128],
                            ident[:R, :R],
                        )
                    nc.vector.tensor_copy(sb[:, :, m0 : m0 + R], wtp[:, :, :R])

            # prefetch the first superchunk's x tiles ahead of the weight
            # DMAs so the transpose pipeline starts immediately.
            xf_pre = []
            for sub in range(4):
                xf = xstage.tile([128, D], f32, name="xf", tag="xf")
                nc.sync.dma_start(out=xf[:, :], in_=x_p[sub * 128 : (sub + 1) * 128, :])
                xf_pre.append(xf)

            build_w(wqk_p, wqk_bf, 2 * DL)
            build_w(wv_p, wv_bf, DL)
            bqk_sb = cpool.tile([128, 2 * DL // 128, 1], f32, name="bqk_sb")
            nc.sync.dma_start(
                out=bqk_sb[:, :, :], in_=bqk_p[:, :].rearrange("(c p) o -> p c o", p=128)
            )
            # v bias as a [1, DL] row broadcast to all 128 partitions (added
            # into v65's value columns during phase 2).
            bvrow = cpool.tile([1, DL], f32, name="bvrow")
            nc.sync.dma_start(out=bvrow[:, :], in_=bv_p[:, :].rearrange("m o -> o m"))
            bvf = cpool.tile([128, DL], f32, name="bvf")
            nc.gpsimd.partition_broadcast(bvf[:, :], bvrow[:, :], channels=128)
            bo0_sb = cpool.tile([128, 1], f32, name="bo0_sb")
            nc.sync.dma_start(out=bo0_sb[:, :], in_=bo_p[0:128, :])
            bo1_sb = cpool.tile([64, 1], f32, name="bo1_sb")
            nc.sync.dma_start(out=bo1_sb[:, :], in_=bo_p[128:DL, :])

            # ---------------- persistent activations ----------------
            qT = cpool.tile([64, HL, S], bf16, name="qT")  # [64, 3, 4096]
            kT = cpool.tile([64, HL, S], bf16, name="kT")
            # v65: per sj-chunk j, per head h: cols h*65..h*65+63 = v values
            # + bv, col h*65+64 = ones (denominator -> po row 64).
            v65 = cpool.tile([128, NKC, HL * (HD + 1)], bf16, name="v65")
            poT = cpool.tile([64, HL, S], bf16, name="poT")  # preout^T per head

            nc.gpsimd.memset(v65[:, :, :], 1.0)

            # ---------------- phase 0-2: x^T, qk^T, v ----------------
            with tc.tile_pool(name="xt", bufs=1) as xtpool:
                xT = xtpool.tile([128, NDC, S], bf16, name="xT")  # 48KB/partition
                for t in range(NSUP):
                    if t == 2:
                        # build wo during superchunk 1: off the ramp critical
                        # path but well before the first output projection.
                        build_w(wo_p, wo_bf, DL)
                    sc = slice(t * SUP, (t + 1) * SUP)
                    for sub in range(4):
                        s0 = t * SUP + sub * 128
                        if t == 0:
                            xf = xf_pre[sub]
                        else:
                            xf = xstage.tile([128, D], f32, name="xf", tag="xf")
                            nc.sync.dma_start(out=xf[:, :], in_=x_p[s0 : s0 + 128, :])
                        xb = xstage.tile([128, D], bf16, name="xb", tag="xb")
                        nc.vector.tensor_copy(xb[:, :], xf[:, :])
                        tp = mmpsum.tile([128, NDC, 128], bf16, name="tp", tag="mm")
                        for dc in range(NDC):
                            nc.tensor.transpose(
                                tp[:, dc, :], xb[:, dc * 128 : (dc + 1) * 128],
                                ident[:, :],
                            )
                        nc.vector.tensor_copy(xT[:, :, s0 : s0 + 128], tp[:, :, :])

                    # qk^T for this superchunk: out [m, s]; drains routed into
                    # the row-tiled qT2/kT2 layout (h2 written to both halves).
                    for mc in range(2 * DL // 128):
                        ps = mmpsum.tile([128, 512], f32, name="ps", tag="mm")
                        for dc in range(NDC):
                            nc.tensor.matmul(
                                ps[:, :],
                                lhsT=wqk_bf[:, dc, mc * 128 : (mc + 1) * 128],
                                rhs=xT[:, dc, sc],
                                start=(dc == 0),
                                stop=(dc == NDC - 1),
                            )
                        for half in (0, 1):
                            g = mc * 128 + half * 64  # global row in [q(192); k(192)]
                            dst = (
                                qT[:, g // 64, sc]
                                if g < DL
                                else kT[:, (g - DL) // 64, sc]
                            )
                            # drain on ScalarE: copy+bias+cast in one ACT op;
                            # keeps the DVE queue clear for attention masks
                            # (GpSimd cannot read PSUM, DVE drains measure
                            # ~50us slower end-to-end).
                            nc.scalar.activation(
                                dst,
                                ps[half * 64 : half * 64 + 64, :],
                                IDENT,
                                bias=bqk_sb[half * 64 : half * 64 + 64, mc, :],
                            )

                    # v for this superchunk (bias folded in here); two sj
                    # chunks share one PSUM tile to halve mm-ring traffic.
                    for sp in range(2):
                        pv = popsum.tile([128, 2, DL], f32, name="pv", tag="po")
                        for i in range(2):
                            j = t * 4 + sp * 2 + i
                            for dc in range(NDC):
                                nc.tensor.matmul(
                                    pv[:, i, :],
                                    lhsT=xT[:, dc, j * 128 : (j + 1) * 128],
                                    rhs=wv_bf[:, dc, :],
                                    start=(dc == 0),
                                    stop=(dc == NDC - 1),
                                )
                        for i in range(2):
                            j = t * 4 + sp * 2 + i
                            nc.vector.tensor_tensor(
                                v65[:, j, :].rearrange("p (h w) -> p h w", h=HL)[
                                    :, :, 0:HD
                                ],
                                pv[:, i, :].rearrange("p (h w) -> p h w", h=HL),
                                bvf[:, :].rearrange("p (h w) -> p h w", h=HL),
                                mybir.AluOpType.add,
                            )

            # ---------------- phase 3: flash attention (logits transposed) ----------------
            def vsl(j, h):  # v65 slice for (chunk j, head h): [128, 65]
                return v65[:, j, :].rearrange("p (hh w) -> p hh w", hh=HL)[:, h, :]

            def normalize(po, h, sc):
                rc = pspool.tile([1, 512], f32, name="rc", tag="rc")
                nc.vector.tensor_copy(rc[:, :], po[64:65, :])
                bcs = bcpool.tile([64, 512], f32, name="bcs", tag="bc")
                nc.gpsimd.partition_broadcast(bcs[:, :], rc[:, :], channels=64)
                nc.vector.reciprocal_approx_fast(out=bcs[:, :], in_=bcs[:, :])
                nc.vector.tensor_mul(poT[:, h, sc], po[0:64, :], bcs[:, :])

            for t in range(NSUP):
                si0 = t * SUP
                sc = slice(si0, si0 + SUP)
                n_j = 4 * t + 4
                for h in range(HL):
                    po = popsum.tile([65, 512], f32, name="po", tag="po")
                    for pr in range(n_j // 2):
                        off = 256 if pr == 2 * t + 1 else 0
                        lg = lgpsum.tile([128, 2, 512], f32, name="lg", tag="lg")
                        aT = atpool.tile([128, 2, 512], bf16, name="aT", tag="at")
                        for half in (0, 1):
                            j = 2 * pr + half
                            sj = slice(128 * j, 128 * (j + 1))
                            nc.tensor.matmul(
                                lg[:, half, off:],
                                lhsT=kT[:, h, sj],
                                rhs=qT[:, h, si0 + off : si0 + SUP],
                                start=True,
                                stop=True,
                            )
                        nc.scalar.activation(
                            aT[:, :, off:], lg[:, :, off:], EXP, scale=0.125
                        )
                        for half in (0, 1):
                            j = 2 * pr + half
                            krel = j - 4 * t
                            if krel >= 0:
                                nc.vector.tensor_mul(
                                    aT[:, half, off:],
                                    aT[:, half, off:],
                                    masks[:, krel, off:],
                                )
                            nc.tensor.matmul(
                                po[:, off:],
                                lhsT=vsl(j, h),
                                rhs=aT[:, half, off:],
                                start=(j == 0),
                                stop=(j == n_j - 1),
                            )
                    normalize(po, h, sc)
                # ---------------- phase 4: chunked AllGather ----------------
                for h in range(HL):
                    nc.sync.dma_start(
                        out=cins[t][HD * h : HD * (h + 1), :],
                        in_=poT[:, h, sc],
                    )
                nc.gpsimd.collective_compute(
                    "AllGather",
                    mybir.AluOpType.bypass,
                    replica_groups=GROUPS,
                    ins=[cins[t][:, :]],
                    outs=[couts[t][:, :]],
                )

            # ---------------- phase 5: output projection (dout-sharded) ----------------
            with tc.tile_pool(name="ccp", bufs=1) as ccpool:
                for c in range(NSUP):
                    strips = []
                    for dc in range(NDC):
                        strip = ccpool.tile(
                            [128, CW], bf16, name=f"ccs{c}_{dc}", tag=f"ccs{dc}", bufs=2
                        )
                        nc.sync.dma_start(
                            out=strip[:, :], in_=couts[c][dc * 128 : (dc + 1) * 128, :]
                        )
                        strips.append(strip)
                    for oc, M0, bo_sb in ((0, 128, bo0_sb), (1, 64, bo1_sb)):
                        pso = mmpsum.tile([128, 512], f32, name="pso", tag="mm")
                        for dc in range(NDC):
                            nc.tensor.matmul(
                                pso[0:M0, :],
                                lhsT=wo_bf[:, dc, oc * 128 : oc * 128 + M0],
                                rhs=strips[dc][:, :],
                                start=(dc == 0),
                                stop=(dc == NDC - 1),
                            )
                        ot = otpool.tile([128, 512], f32, name="ot", tag="ot")
                        nc.vector.tensor_scalar_add(
                            ot[0:M0, :], pso[0:M0, :], bo_sb[:, :]
                        )
                        nc.sync.dma_start(
                            out=out_p[
                                oc * 128 : oc * 128 + M0, c * SUP : (c + 1) * SUP
                            ],
                            in_=ot[0:M0, :],
                        )

    nc.finalize()
    return nc


def _get_nc():
    if "nc" not in _CACHE:
        _CACHE["nc"] = _build_nc()
    return _CACHE["nc"]


def _make_in_maps(x, Wq_w, Wq_b, Wk_w, Wk_b, Wv_w, Wv_b, Wo_w, Wo_b):
    f = np.float32
    in_maps = []
    for c in range(NCORES):
        b, hg = divmod(c, 4)
        r = slice(hg * DL, (hg + 1) * DL)
        in_maps.append(
            {
                "x": np.ascontiguousarray(x[b], dtype=f),
                "wqk": np.ascontiguousarray(
                    np.concatenate([Wq_w[r], Wk_w[r]], axis=0), dtype=f
                ),
                "bqk": np.ascontiguousarray(
                    np.concatenate([Wq_b[r], Wk_b[r]])[:, None], dtype=f
                ),
                "wv": np.ascontiguousarray(Wv_w[r], dtype=f),
                "bv": np.ascontiguousarray(Wv_b[r][:, None], dtype=f),
                "wo": np.ascontiguousarray(Wo_w[r], dtype=f),
                "bo": np.ascontiguousarray(Wo_b[r][:, None], dtype=f),
            }
        )
    return in_maps


def run_on_hw(in_maps, trace=False):
    from concourse.bass_utils import run_bass_kernel_spmd

    nc = _get_nc()
    return run_bass_kernel_spmd(nc, in_maps, core_ids=list(range(NCORES)), trace=trace)


def kernel(x, Wq_w, Wq_b, Wk_w, Wk_b, Wv_w, Wv_b, Wo_w, Wo_b):
    in_maps = _make_in_maps(
        np.asarray(x, dtype=np.float32),
        *[
            np.asarray(a, dtype=np.float32)
            for a in (Wq_w, Wq_b, Wk_w, Wk_b, Wv_w, Wv_b, Wo_w, Wo_b)
        ],
    )
    res = run_on_hw(in_maps, trace=False)
    out = np.empty((B, S, D), dtype=np.float32)
    for c in range(NCORES):
        b, hg = divmod(c, 4)
        out[b, :, hg * DL : (hg + 1) * DL] = res.results[c]["out"].T
    return out
